# revision 7
# baseline (speedup 1.0000x reference)
"""CapsuleLayer (dynamic routing) Trainium2 Bass kernel.

Math (per example b):
  u_hat[b,i,o,n] = sum_v x[b,i,v] * W[i,o,v,n]        I=1152, O=10, V=8, N=16
  b_logits = 0; repeat n_routing times:
    c = softmax_o(b_logits); s = sum_i c*u_hat; out = squash(s)
    if not last: b_logits += sum_n u_hat*out

Distribution: batch B=256 sharded over 8 cores (32 each). W replicated.

Per-core kernel layout (chunk = 8 examples, 4 chunks), i = ib*16 + il:
  K partitions k = il*8+v   (contraction rows of the u_hat matmul)
  M partitions p = b*16+il  (rows of u_hat / routing state)
  U   [128, 72, 160]  u_hat chunk,  U[(b,il), ib, (o,n)]
  XBD [128, 128, 72]  block-diag x stationary: XBD[(il,v),(b,il'),ib], il'==il
  CBD [128,  80, 72]  block-diag c stationary: CBD[(b,il),(o,b'),ib], b'==b
  w2  [128, 72, 160]  W2[(il,v), ib, (o,n)] = W[ib*16+il, o, v, n] (host-prep)
  u_hat matmul (per ib): psum[(b,il'),(o,n)] = XBD[:,:,ib].T @ w2[:,ib,:]
  s matmul (per iter): psum[(o,b'),(o',n)] += CBD[:,:,ib].T @ U[:,ib,:]
    -> diagonal o==o' holds s[b', o, n]
Block-diagonal scatters are plain DMAs (ib innermost => 288B runs, contiguous
partition ranges only — neither walrus nor the sim likes strided partitions).
"""

import os
import sys

import numpy as np

_TRN_REPO = "/opt/trn_rl_repo"
if _TRN_REPO not in sys.path:
    sys.path.insert(0, _TRN_REPO)

EPS = 1e-10
B, I, V, O, N = 256, 1152, 8, 10, 16
NCORES = 8
BLOC = B // NCORES          # 32 examples per core
BC = 8                      # examples per chunk
NCHUNK = BLOC // BC         # 4
IB = I // 16                # 72 i-blocks
ON = O * N                  # 160


def _build(n_routing: int):
    import concourse.bacc as bacc
    import concourse.tile as tile
    from concourse import mybir

    nc = bacc.Bacc("TRN2", target_bir_lowering=False, debug=False)
    f32 = mybir.dt.float32

    xd = nc.dram_tensor("xd", [16, 8, BLOC, IB], f32, kind="ExternalInput")
    w2 = nc.dram_tensor("w2", [128, IB, ON], f32, kind="ExternalInput")
    e2 = nc.dram_tensor("e2", [128, 80], f32, kind="ExternalInput")
    out_d = nc.dram_tensor("out", [BLOC, O, N], f32, kind="ExternalOutput")

    AX = mybir.AxisListType
    OP = mybir.AluOpType
    AF = mybir.ActivationFunctionType

    with tile.TileContext(nc) as tc:
        with (
            tc.tile_pool(name="singles", bufs=1) as singles,
            tc.tile_pool(name="small", bufs=2) as small,
            tc.tile_pool(name="prodp", bufs=1) as prodp,
            tc.tile_pool(name="psA", bufs=4, space="PSUM") as psA,
            tc.tile_pool(name="psS", bufs=2, space="PSUM") as psS,
        ):
            w2s = singles.tile([128, IB, ON], f32)
            nc.sync.dma_start(out=w2s[:], in_=w2[:])
            e2s = singles.tile([128, 80], f32)
            nc.sync.dma_start(out=e2s[:], in_=e2[:])

            xbd = singles.tile([128, 128, IB], f32)
            nc.vector.memset(xbd[:], 0.0)
            cbd = singles.tile([128, 80, IB], f32)
            if n_routing > 1:
                nc.vector.memset(cbd[:], 0.0)

            U = singles.tile([128, IB, ON], f32)
            vrep = singles.tile([128, ON], f32)

            for c in range(NCHUNK):
                # ---- scatter x chunk into block-diagonal stationary ----
                # dest m-slots (b, il'=il): free positions il, il+16, ...
                for il in range(16):
                    nc.sync.dma_start(
                        out=xbd[il * 8:(il + 1) * 8, il:128:16, :],
                        in_=xd[il, :, c * BC:(c + 1) * BC, :],
                    )
                # ---- u_hat: 72 matmuls, copies to SBUF in groups of 3 ----
                for g in range(IB // 3):
                    ps = psA.tile([128, 3, ON], f32, tag="psA")
                    for j in range(3):
                        ib = g * 3 + j
                        nc.tensor.matmul(
                            ps[:, j, :],
                            xbd[:, :, ib],
                            w2s[:, ib, :],
                            start=True,
                            stop=True,
                        )
                    if g % 2 == 0:
                        nc.vector.tensor_copy(U[:, g * 3:(g + 1) * 3, :], ps[:])
                    else:
                        nc.scalar.copy(U[:, g * 3:(g + 1) * 3, :], ps[:])

                # ---- routing iterations ----
                bb = None
                for it in range(n_routing):
                    # s accumulation over i-blocks
                    pss = psS.tile([80, ON], f32, tag="psS")
                    for ib in range(IB):
                        lhsT = e2s[:] if it == 0 else cbd[:, :, ib]
                        nc.tensor.matmul(
                            pss[:],
                            lhsT,
                            U[:, ib, :],
                            start=(ib == 0),
                            stop=(ib == IB - 1),
                        )
                    # PSUM -> SBUF, then extract diag s[b, (o,n)] via DMAs
                    # (engine APs must start at partition 0/32/64/96; DMAs
                    # are exempt, so the o*8-based slices go through DMA)
                    sY = small.tile([80, ON], f32, tag="sY")
                    nc.scalar.copy(sY[:], pss[:])
                    s3 = small.tile([BC, ON], f32, tag="s3")
                    for o in range(O):
                        nc.sync.dma_start(
                            out=s3[:, o * N:(o + 1) * N],
                            in_=sY[o * 8:(o + 1) * 8, o * N:(o + 1) * N],
                        )
                    # squash: v3 = s3 * nsq/((1+nsq)*sqrt(nsq+eps)), per (b,o)
                    sq = small.tile([BC, ON], f32, tag="sq")
                    nc.scalar.activation(sq[:], s3[:], AF.Square)
                    nsq = small.tile([BC, O], f32, tag="nsq")
                    nc.vector.tensor_reduce(
                        nsq[:],
                        sq[:].rearrange("b (o n) -> b o n", n=N),
                        axis=AX.X,
                        op=OP.add,
                    )
                    nse = small.tile([BC, O], f32, tag="nse")
                    nc.vector.tensor_scalar_add(nse[:], nsq[:], EPS)
                    rt = small.tile([BC, O], f32, tag="rt")
                    nc.scalar.activation(rt[:], nse[:], AF.Sqrt)
                    # one Newton step for sqrt accuracy: y = .5*(rt + nse/rt)
                    r0 = small.tile([BC, O], f32, tag="r0")
                    nc.vector.reciprocal(r0[:], rt[:])
                    t0 = small.tile([BC, O], f32, tag="t0")
                    nc.vector.tensor_mul(t0[:], nse[:], r0[:])
                    y0 = small.tile([BC, O], f32, tag="y0")
                    nc.vector.tensor_add(y0[:], rt[:], t0[:])
                    y1 = small.tile([BC, O], f32, tag="y1")
                    nc.vector.tensor_scalar_mul(y1[:], y0[:], 0.5)
                    np1 = small.tile([BC, O], f32, tag="np1")
                    nc.vector.tensor_scalar_add(np1[:], nsq[:], 1.0)
                    dd = small.tile([BC, O], f32, tag="dd")
                    nc.vector.tensor_mul(dd[:], np1[:], y1[:])
                    rd = small.tile([BC, O], f32, tag="rd")
                    nc.vector.reciprocal(rd[:], dd[:])
                    sc = small.tile([BC, O], f32, tag="sc")
                    nc.vector.tensor_mul(sc[:], nsq[:], rd[:])
                    v3 = small.tile([32, ON], f32, tag="v3")
                    nc.vector.memset(v3[:], 0.0)
                    nc.vector.tensor_mul(
                        v3[0:BC, :].rearrange("b (o n) -> b o n", n=N),
                        s3[:].rearrange("b (o n) -> b o n", n=N),
                        sc[:].unsqueeze(2).broadcast_to([BC, O, N]),
                    )

                    if it == n_routing - 1:
                        # final output for this chunk
                        nc.sync.dma_start(
                            out=out_d[c * BC:(c + 1) * BC, :, :],
                            in_=v3[0:BC, :].rearrange("b (o n) -> b o n", n=N),
                        )
                        continue

                    # replicate v across il: vrep[(b,il), (o,n)] = v[b,o,n]
                    for q in range(4):
                        nc.vector.stream_shuffle(
                            vrep[q * 32:(q + 1) * 32, :],
                            v3[:],
                            [2 * q + (j // 16) for j in range(32)],
                        )
                    # agreement a[(b,il), ib, o] = sum_n U * vrep (two halves)
                    a_t = small.tile([128, IB, O], f32, tag=f"a{it}")
                    H = IB // 2
                    for h in range(2):
                        ph = prodp.tile([128, H, ON], f32, tag="prodh")
                        nc.vector.tensor_mul(
                            ph[:].rearrange("p i (o n) -> p i o n", n=N),
                            U[:, h * H:(h + 1) * H, :].rearrange(
                                "p i (o n) -> p i o n", n=N
                            ),
                            vrep[:]
                            .rearrange("p (o n) -> p o n", n=N)
                            .unsqueeze(1)
                            .broadcast_to([128, H, O, N]),
                        )
                        nc.vector.tensor_reduce(
                            a_t[:, h * H:(h + 1) * H, :],
                            ph[:].rearrange("p i (o n) -> p i o n", n=N),
                            axis=AX.X,
                            op=OP.add,
                        )
                    if bb is None:
                        bb = a_t
                    else:
                        bb2 = small.tile([128, IB, O], f32, tag=f"bb{it}")
                        nc.vector.tensor_add(bb2[:], bb[:], a_t[:])
                        bb = bb2

                    # softmax over o -> c2n [128, O, IB], then scatter to CBD
                    c2 = small.tile([128, O, IB], f32, tag="c2")
                    nc.scalar.activation(
                        c2[:].transpose([0, 2, 1]), bb[:], AF.Exp
                    )
                    ssum = small.tile([128, IB], f32, tag="ssum")
                    nc.vector.tensor_reduce(
                        ssum[:], c2[:].transpose([0, 2, 1]), axis=AX.X, op=OP.add
                    )
                    rs = small.tile([128, IB], f32, tag="rs")
                    nc.vector.reciprocal(rs[:], ssum[:])
                    c2n = small.tile([128, O, IB], f32, tag="c2n")
                    nc.vector.tensor_mul(
                        c2n[:], c2[:], rs[:].unsqueeze(1).broadcast_to([128, O, IB])
                    )
                    for b in range(BC):
                        nc.sync.dma_start(
                            out=cbd[b * 16:(b + 1) * 16, b:80:8, :],
                            in_=c2n[b * 16:(b + 1) * 16, :, :],
                        )

    nc.compile()
    return nc


_CACHE = {}


def _get(n_routing: int):
    if n_routing not in _CACHE:
        _CACHE[n_routing] = _build(n_routing)
    return _CACHE[n_routing]


def _prep_host(inputs: np.ndarray, W: np.ndarray):
    x = np.ascontiguousarray(np.asarray(inputs, dtype=np.float32))
    W = np.asarray(W, dtype=np.float32)
    # w2[(il,v), ib, (o,n)] = W[ib*16+il, o, v, n]
    w2 = np.ascontiguousarray(
        W.reshape(IB, 16, O, V, N).transpose(1, 3, 0, 2, 4).reshape(128, IB, ON)
    )
    # e2[(b,il), (o,b')] = 0.1 * (b == b')   (uniform softmax weights)
    e2 = np.zeros((128, 80), dtype=np.float32)
    for b in range(8):
        e2[b * 16:(b + 1) * 16, np.arange(O) * 8 + b] = 0.1
    return x, w2, e2


def kernel(inputs, W, n_routing):
    from concourse.bass_utils import run_bass_kernel_spmd

    n_routing = int(n_routing)
    nc = _get(n_routing)
    x, w2, e2 = _prep_host(inputs, W)

    in_maps = []
    for core in range(NCORES):
        xc = x[core * BLOC:(core + 1) * BLOC]              # [32, 1152, 8]
        # xd[il, v, b, ib] = xc[b, ib*16+il, v]
        xdc = np.ascontiguousarray(
            xc.reshape(BLOC, IB, 16, V).transpose(2, 3, 0, 1)
        )
        in_maps.append({"xd": xdc, "w2": w2, "e2": e2})

    res = run_bass_kernel_spmd(nc, in_maps, core_ids=list(range(NCORES)))
    outs = [res.results[i]["out"] for i in range(NCORES)]
    return np.concatenate(outs, axis=0).astype(np.float32)


# revision 9
# speedup vs baseline: 1.5763x; 1.5763x over previous
"""CapsuleLayer (dynamic routing) Trainium2 Bass kernel.

Math (per example b):
  u_hat[b,i,o,n] = sum_v x[b,i,v] * W[i,o,v,n]        I=1152, O=10, V=8, N=16
  b_logits = 0; repeat n_routing times:
    c = softmax_o(b_logits); s = sum_i c*u_hat; out = squash(s)
    if not last: b_logits += sum_n u_hat*out

Distribution: batch B=256 sharded over 8 cores (32 each). W replicated.

Per-core kernel layout (chunk = 8 examples, 4 chunks), i = ib*16 + il:
  K partitions k = il*8+v   (contraction rows of the u_hat matmul)
  M partitions p = b*16+il  (rows of u_hat / routing state)
  U   [128, 72, 160] bf16  u_hat chunk,  U[(b,il), ib, (o,n)]
  XBD [128, 128, 72] bf16  block-diag x stationary: XBD[(il,v),(b,il'),ib]
  CBD [128, 128, 72] bf16  block-diag c stationary: CBD[(b,il),(o,b'),ib]
                           (m padded 80->128 so FWL fast-weight-load kicks in)
  w2  [128, 72, 160] bf16  W2[(il,v), ib, (o,n)] = W[ib*16+il, o, v, n]
  u_hat matmul (per ib): psum[(b,il'),(o,n)] = XBD[:,:,ib].T @ w2[:,ib,:]
  s matmul (per iter): psum[(o,b'),(o',n)] += CBD[:,:,ib].T @ U[:,ib,:]
    -> diagonal o==o' holds s[b', o, n]  (extracted via small DMAs)
Everything on the matmul path is bf16 (fp32 matmuls lower to 2 PE passes);
s-accumulation, squash and softmax statistics stay fp32.
Block-diagonal scatters are plain DMAs (ib innermost => contiguous runs,
contiguous partition ranges only).
"""

import os
import sys

import numpy as np

_TRN_REPO = "/opt/trn_rl_repo"
if _TRN_REPO not in sys.path:
    sys.path.insert(0, _TRN_REPO)

EPS = 1e-10
B, I, V, O, N = 256, 1152, 8, 10, 16
NCORES = 8
BLOC = B // NCORES          # 32 examples per core
BC = 8                      # examples per chunk
NCHUNK = BLOC // BC         # 4
IB = I // 16                # 72 i-blocks
ON = O * N                  # 160


def _build(n_routing: int):
    import concourse.bacc as bacc
    import concourse.tile as tile
    from concourse import mybir

    nc = bacc.Bacc("TRN2", target_bir_lowering=False, debug=False)
    f32 = mybir.dt.float32
    bf16 = mybir.dt.bfloat16

    xd = nc.dram_tensor("xd", [16, 8, BLOC, IB], bf16, kind="ExternalInput")
    w2 = nc.dram_tensor("w2", [128, IB, ON], bf16, kind="ExternalInput")
    e2 = nc.dram_tensor("e2", [128, 128], bf16, kind="ExternalInput")
    out_d = nc.dram_tensor("out", [BLOC, O, N], f32, kind="ExternalOutput")

    AX = mybir.AxisListType
    OP = mybir.AluOpType
    AF = mybir.ActivationFunctionType

    with tile.TileContext(nc) as tc:
        with (
            tc.tile_pool(name="singles", bufs=1) as singles,
            tc.tile_pool(name="small", bufs=2) as small,
            tc.tile_pool(name="prodp", bufs=1) as prodp,
            tc.tile_pool(name="psA", bufs=4, space="PSUM") as psA,
            tc.tile_pool(name="psS", bufs=2, space="PSUM") as psS,
        ):
            w2s = singles.tile([128, IB, ON], bf16)
            nc.sync.dma_start(out=w2s[:], in_=w2[:])
            e2s = singles.tile([128, 128], bf16)
            nc.sync.dma_start(out=e2s[:], in_=e2[:])

            xbd = singles.tile([128, 128, IB], bf16)
            nc.vector.memset(xbd[:], 0.0)
            cbd = singles.tile([128, 128, IB], bf16)
            if n_routing > 1:
                nc.vector.memset(cbd[:], 0.0)

            U = singles.tile([128, IB, ON], bf16)
            vrep = singles.tile([128, ON], bf16)

            for c in range(NCHUNK):
                # ---- scatter x chunk into block-diagonal stationary ----
                # dest m-slots (b, il'=il): free positions il, il+16, ...
                for il in range(16):
                    nc.sync.dma_start(
                        out=xbd[il * 8:(il + 1) * 8, il:128:16, :],
                        in_=xd[il, :, c * BC:(c + 1) * BC, :],
                    )
                # ---- u_hat: 72 matmuls, copies to SBUF in groups of 3 ----
                for g in range(IB // 3):
                    ps = psA.tile([128, 3, ON], f32, tag="psA")
                    for j in range(3):
                        ib = g * 3 + j
                        nc.tensor.matmul(
                            ps[:, j, :],
                            xbd[:, :, ib],
                            w2s[:, ib, :],
                            start=True,
                            stop=True,
                        )
                    if g % 2 == 0:
                        nc.vector.tensor_copy(U[:, g * 3:(g + 1) * 3, :], ps[:])
                    else:
                        nc.scalar.copy(U[:, g * 3:(g + 1) * 3, :], ps[:])

                # ---- routing iterations ----
                bb = None
                for it in range(n_routing):
                    # s accumulation over i-blocks (only 80 of 128 PSUM rows used)
                    pss = psS.tile([128, ON], f32, tag="psS")
                    for ib in range(IB):
                        lhsT = e2s[:] if it == 0 else cbd[:, :, ib]
                        nc.tensor.matmul(
                            pss[:],
                            lhsT,
                            U[:, ib, :],
                            start=(ib == 0),
                            stop=(ib == IB - 1),
                        )
                    # PSUM -> SBUF, then extract diag s[b, (o,n)] via DMAs
                    # (engine APs must start at partition 0/32/64/96; DMAs
                    # are exempt, so the o*8-based slices go through DMA)
                    sY = small.tile([80, ON], f32, tag="sY")
                    nc.scalar.copy(sY[:], pss[0:80, :])
                    s3 = small.tile([BC, ON], f32, tag="s3")
                    for o in range(O):
                        nc.scalar.dma_start(
                            out=s3[:, o * N:(o + 1) * N],
                            in_=sY[o * 8:(o + 1) * 8, o * N:(o + 1) * N],
                        )
                    # squash: v3 = s3 * nsq/((1+nsq)*sqrt(nsq+eps)), per (b,o)
                    sq = small.tile([BC, ON], f32, tag="sq")
                    nc.vector.tensor_mul(sq[:], s3[:], s3[:])
                    nsq = small.tile([BC, O], f32, tag="nsq")
                    nc.vector.tensor_reduce(
                        nsq[:],
                        sq[:].rearrange("b (o n) -> b o n", n=N),
                        axis=AX.X,
                        op=OP.add,
                    )
                    nse = small.tile([BC, O], f32, tag="nse")
                    nc.vector.tensor_scalar_add(nse[:], nsq[:], EPS)
                    # sqrt via exp(0.5*ln(x)) keeps ACT on one table set
                    # (exp is needed for softmax); Newton step restores acc.
                    lnx = small.tile([BC, O], f32, tag="lnx")
                    nc.scalar.activation(lnx[:], nse[:], AF.Ln)
                    rt = small.tile([BC, O], f32, tag="rt")
                    nc.scalar.activation(rt[:], lnx[:], AF.Exp, scale=0.5)
                    # Newton: y = .5*(rt + nse/rt)
                    r0 = small.tile([BC, O], f32, tag="r0")
                    nc.vector.reciprocal(r0[:], rt[:])
                    t0 = small.tile([BC, O], f32, tag="t0")
                    nc.vector.tensor_mul(t0[:], nse[:], r0[:])
                    y0 = small.tile([BC, O], f32, tag="y0")
                    nc.vector.tensor_add(y0[:], rt[:], t0[:])
                    y1 = small.tile([BC, O], f32, tag="y1")
                    nc.vector.tensor_scalar_mul(y1[:], y0[:], 0.5)
                    np1 = small.tile([BC, O], f32, tag="np1")
                    nc.vector.tensor_scalar_add(np1[:], nsq[:], 1.0)
                    dd = small.tile([BC, O], f32, tag="dd")
                    nc.vector.tensor_mul(dd[:], np1[:], y1[:])
                    rd = small.tile([BC, O], f32, tag="rd")
                    nc.vector.reciprocal(rd[:], dd[:])
                    sc = small.tile([BC, O], f32, tag="sc")
                    nc.vector.tensor_mul(sc[:], nsq[:], rd[:])
                    v3 = small.tile([32, ON], f32, tag="v3")
                    nc.vector.memset(v3[:], 0.0)
                    nc.vector.tensor_mul(
                        v3[0:BC, :].rearrange("b (o n) -> b o n", n=N),
                        s3[:].rearrange("b (o n) -> b o n", n=N),
                        sc[:].unsqueeze(2).broadcast_to([BC, O, N]),
                    )

                    if it == n_routing - 1:
                        # final output for this chunk
                        nc.scalar.dma_start(
                            out=out_d[c * BC:(c + 1) * BC, :, :],
                            in_=v3[0:BC, :].rearrange("b (o n) -> b o n", n=N),
                        )
                        continue

                    # replicate v across il: vrep[(b,il), (o,n)] = v[b,o,n]
                    v3b = small.tile([32, ON], bf16, tag="v3b")
                    nc.vector.tensor_copy(v3b[:], v3[:])
                    for q in range(4):
                        nc.vector.stream_shuffle(
                            vrep[q * 32:(q + 1) * 32, :],
                            v3b[:],
                            [2 * q + (j // 16) for j in range(32)],
                        )
                    # agreement a[(b,il), ib, o] = sum_n U * vrep (two halves)
                    a_t = small.tile([128, IB, O], f32, tag=f"a{it}")
                    H = IB // 2
                    for h in range(2):
                        ph = prodp.tile([128, H, ON], bf16, tag="prodh")
                        nc.vector.tensor_mul(
                            ph[:].rearrange("p i (o n) -> p i o n", n=N),
                            U[:, h * H:(h + 1) * H, :].rearrange(
                                "p i (o n) -> p i o n", n=N
                            ),
                            vrep[:]
                            .rearrange("p (o n) -> p o n", n=N)
                            .unsqueeze(1)
                            .broadcast_to([128, H, O, N]),
                        )
                        nc.vector.tensor_reduce(
                            a_t[:, h * H:(h + 1) * H, :],
                            ph[:].rearrange("p i (o n) -> p i o n", n=N),
                            axis=AX.X,
                            op=OP.add,
                        )
                    if bb is None:
                        bb = a_t
                    else:
                        bb2 = small.tile([128, IB, O], f32, tag=f"bb{it}")
                        nc.vector.tensor_add(bb2[:], bb[:], a_t[:])
                        bb = bb2

                    # softmax over o -> c2n [128, O, IB] bf16, scatter to CBD
                    c2 = small.tile([128, O, IB], f32, tag="c2")
                    nc.scalar.activation(
                        c2[:].transpose([0, 2, 1]), bb[:], AF.Exp
                    )
                    ssum = small.tile([128, IB], f32, tag="ssum")
                    nc.vector.tensor_reduce(
                        ssum[:], c2[:].transpose([0, 2, 1]), axis=AX.X, op=OP.add
                    )
                    rs = small.tile([128, IB], f32, tag="rs")
                    nc.vector.reciprocal(rs[:], ssum[:])
                    c2n = small.tile([128, O, IB], bf16, tag="c2n")
                    nc.vector.tensor_mul(
                        c2n[:], c2[:], rs[:].unsqueeze(1).broadcast_to([128, O, IB])
                    )
                    for b in range(BC):
                        nc.sync.dma_start(
                            out=cbd[b * 16:(b + 1) * 16, b:80:8, :],
                            in_=c2n[b * 16:(b + 1) * 16, :, :],
                        )

    nc.compile()
    return nc


_CACHE = {}


def _get(n_routing: int):
    if n_routing not in _CACHE:
        _CACHE[n_routing] = _build(n_routing)
    return _CACHE[n_routing]


def _bf16(a):
    import ml_dtypes

    return np.asarray(a, dtype=ml_dtypes.bfloat16)


def _prep_host(inputs: np.ndarray, W: np.ndarray):
    x = np.ascontiguousarray(np.asarray(inputs, dtype=np.float32))
    W = np.asarray(W, dtype=np.float32)
    # w2[(il,v), ib, (o,n)] = W[ib*16+il, o, v, n]
    w2 = np.ascontiguousarray(
        W.reshape(IB, 16, O, V, N).transpose(1, 3, 0, 2, 4).reshape(128, IB, ON)
    )
    # e2[(b,il), (o,b')] = 0.1 * (b == b')   (uniform softmax weights)
    e2 = np.zeros((128, 128), dtype=np.float32)
    for b in range(8):
        e2[b * 16:(b + 1) * 16, np.arange(O) * 8 + b] = 0.1
    return x, _bf16(w2), _bf16(e2)


def _make_in_maps(inputs, W):
    x, w2, e2 = _prep_host(inputs, W)
    in_maps = []
    for core in range(NCORES):
        xc = x[core * BLOC:(core + 1) * BLOC]              # [32, 1152, 8]
        # xd[il, v, b, ib] = xc[b, ib*16+il, v]
        xdc = np.ascontiguousarray(
            _bf16(xc.reshape(BLOC, IB, 16, V).transpose(2, 3, 0, 1))
        )
        in_maps.append({"xd": xdc, "w2": w2, "e2": e2})
    return in_maps


def kernel(inputs, W, n_routing):
    from concourse.bass_utils import run_bass_kernel_spmd

    n_routing = int(n_routing)
    nc = _get(n_routing)
    in_maps = _make_in_maps(inputs, W)
    res = run_bass_kernel_spmd(nc, in_maps, core_ids=list(range(NCORES)))
    outs = [res.results[i]["out"] for i in range(NCORES)]
    return np.concatenate(outs, axis=0).astype(np.float32)


# revision 14
# speedup vs baseline: 2.3755x; 1.5070x over previous
"""CapsuleLayer (dynamic routing) Trainium2 Bass kernel.

Math (per example b):
  u_hat[b,i,o,n] = sum_v x[b,i,v] * W[i,o,v,n]        I=1152, O=10, V=8, N=16
  b_logits = 0; repeat n_routing times:
    c = softmax_o(b_logits); s = sum_i c*u_hat; out = squash(s)
    if not last: b_logits += sum_n u_hat*out

Distribution: batch B=256 sharded over 8 cores (32 each). W replicated.

Per-core layout (chunk = 8 examples, 4 chunks), i = ib*16 + il:
  K partitions k = il*8+v   (contraction rows of the u_hat matmul)
  M partitions p = b*16+il  (rows of u_hat / routing state)
  U[c] [128, 72, 160] bf16  u_hat,  U[(b,il), ib, (o,n)]
  XBD  [128, 128, 72] bf16  block-diag x stationary: XBD[(il,v),(b,il'),ib]
  CBD[c] [128, 80, 72] bf16 block-diag c stationary: CBD[(b,il),(o,b'),ib]
  w2   [128, 72, 160] bf16  W2[(il,v), ib, (o,n)] = W[ib*16+il, o, v, n]
  u_hat matmul (per ib): psum[(b,il'),(o,n)] = XBD[:,:,ib].T @ w2[:,ib,:]
  s matmul (per iter): psum[(o,b'),(o',n)] += CBD[:,:,ib].T @ U[:,ib,:]
    -> diagonal o==o' holds s[b', o, n]  (extracted via small DMAs)

Structure: phase 1 builds u_hat for all 4 chunks (then releases x/W SBUF);
phase 2 runs routing with iterations outer / chunks inner so independent
chunks pipeline across PE (s-matmuls), DVE (agreement) and ACT (copies).
Matmul path all bf16 (fp32 matmuls lower to 2 PE passes); s-accumulation,
squash and softmax statistics stay fp32. Squash uses a DVE-only rsqrt
(bit hack + Newton) so ACT never switches LUT table sets (exp only).
Block-diagonal scatters are plain DMAs (ib innermost => contiguous runs,
contiguous partition ranges only).
"""

import os
import sys

import numpy as np

_TRN_REPO = "/opt/trn_rl_repo"
if _TRN_REPO not in sys.path:
    sys.path.insert(0, _TRN_REPO)

EPS = 1e-10
B, I, V, O, N = 256, 1152, 8, 10, 16
NCORES = 8
BLOC = B // NCORES          # 32 examples per core
BC = 8                      # examples per chunk
NCHUNK = BLOC // BC         # 4
IB = I // 16                # 72 i-blocks
ON = O * N                  # 160
RSQRT_MAGIC = 0x5F3759DF


def _build(n_routing: int):
    import concourse.bacc as bacc
    import concourse.tile as tile
    from concourse import mybir

    nc = bacc.Bacc("TRN2", target_bir_lowering=False, debug=False)
    f32 = mybir.dt.float32
    bf16 = mybir.dt.bfloat16
    i32 = mybir.dt.int32

    xd = nc.dram_tensor("xd", [16, 8, BLOC, IB], bf16, kind="ExternalInput")
    w2 = nc.dram_tensor("w2", [128, IB, ON], bf16, kind="ExternalInput")
    e2 = nc.dram_tensor("e2", [128, 80], bf16, kind="ExternalInput")
    out_d = nc.dram_tensor("out", [BLOC, O, N], f32, kind="ExternalOutput")

    AX = mybir.AxisListType
    OP = mybir.AluOpType
    AF = mybir.ActivationFunctionType

    NPAIR = NCHUNK // 2

    with tile.TileContext(nc) as tc:
        with (
            tc.tile_pool(name="state", bufs=1) as state,
            tc.tile_pool(name="small", bufs=2) as small,
            tc.tile_pool(name="tree", bufs=1) as tree,
            tc.tile_pool(name="psA", bufs=4, space="PSUM") as psA,
            tc.tile_pool(name="psS", bufs=2, space="PSUM") as psS,
        ):
            Us = [
                state.tile([128, IB, ON], bf16, tag=f"U{j}", name=f"U{j}")
                for j in range(2)
            ]
            cbds = [
                state.tile([128, 80, IB], bf16, tag=f"cbd{j}", name=f"cbd{j}")
                for j in range(2)
            ] if n_routing > 1 else []
            bbs = [
                state.tile([128, IB, O], f32, tag=f"bb{j}", name=f"bb{j}")
                for j in range(2)
            ]
            e2s = state.tile([128, 80], bf16)
            nc.sync.dma_start(out=e2s[:], in_=e2[:])
            for cb in cbds:
                nc.vector.memset(cb[:], 0.0)
            w2s = state.tile([128, IB, ON], bf16)
            nc.sync.dma_start(out=w2s[:], in_=w2[:])
            xbd = state.tile([128, 128, IB], bf16)
            nc.vector.memset(xbd[:], 0.0)

            for pair in range(NPAIR):
                cs = [2 * pair, 2 * pair + 1]
                # ------------- phase 1: u_hat for this pair -------------
                for c in cs:
                    for il in range(16):
                        nc.sync.dma_start(
                            out=xbd[il * 8:(il + 1) * 8, il:128:16, :],
                            in_=xd[il, :, c * BC:(c + 1) * BC, :],
                        )
                    for g in range(IB // 3):
                        ps = psA.tile([128, 3, ON], f32, tag="psA")
                        for j in range(3):
                            ib = g * 3 + j
                            nc.tensor.matmul(
                                ps[:, j, :],
                                xbd[:, :, ib],
                                w2s[:, ib, :],
                                start=True,
                                stop=True,
                            )
                        nc.scalar.copy(Us[c % 2][:, g * 3:(g + 1) * 3, :], ps[:])

                # ------------- phase 2: routing for this pair -------------
                for it in range(n_routing):
                    for c in cs:
                        _routing_iter(
                            nc, tc, mybir, small, tree, psS,
                            Us[c % 2],
                            cbds[c % 2] if cbds else None,
                            bbs[c % 2],
                            e2s, out_d, c, it, n_routing,
                        )

    nc.compile()
    return nc


def _routing_iter(nc, tc, mybir, small, tree, psS, U, cbd, bbst, e2s,
                  out_d, c, it, n_routing):
    f32 = mybir.dt.float32
    bf16 = mybir.dt.bfloat16
    i32 = mybir.dt.int32
    AX = mybir.AxisListType
    OP = mybir.AluOpType
    AF = mybir.ActivationFunctionType

    # s accumulation over i-blocks
    pss = psS.tile([80, ON], f32, tag="psS")
    for ib in range(IB):
        lhsT = e2s[:] if it == 0 else cbd[:, :, ib]
        nc.tensor.matmul(
            pss[:], lhsT, U[:, ib, :], start=(ib == 0), stop=(ib == IB - 1)
        )
    # PSUM -> SBUF, extract diag s[b, (o,n)] via DMAs (engine APs must start
    # at partition 0/32/64/96; DMAs are exempt from the base rule)
    sY = small.tile([80, ON], f32, tag="sY")
    nc.scalar.copy(sY[:], pss[:])
    s3 = small.tile([BC, ON], f32, tag="s3")
    for o in range(O):
        nc.scalar.dma_start(
            out=s3[:, o * N:(o + 1) * N],
            in_=sY[o * 8:(o + 1) * 8, o * N:(o + 1) * N],
        )
    # squash: v3 = s3 * nsq * rsqrt(nse*(1+nsq)^2), fp32, DVE-only
    sq = small.tile([BC, ON], f32, tag="sq")
    nc.vector.tensor_mul(sq[:], s3[:], s3[:])
    nsq = small.tile([BC, O], f32, tag="nsq")
    nc.vector.tensor_reduce(
        nsq[:], sq[:].rearrange("b (o n) -> b o n", n=N), axis=AX.X, op=OP.add
    )
    np1 = small.tile([BC, O], f32, tag="np1")
    nc.vector.tensor_scalar_add(np1[:], nsq[:], 1.0)
    d1 = small.tile([BC, O], f32, tag="d1")
    nc.vector.tensor_mul(d1[:], np1[:], np1[:])
    nse = small.tile([BC, O], f32, tag="nse")
    nc.vector.tensor_scalar_add(nse[:], nsq[:], EPS)
    dd = small.tile([BC, O], f32, tag="dd")
    nc.vector.tensor_mul(dd[:], d1[:], nse[:])
    # rsqrt(dd): bit hack + 3 Newton steps (all DVE, no ACT table switch)
    yy = small.tile([BC, O], f32, tag="yy")
    nc.vector.tensor_scalar(
        yy[:].bitcast(i32), dd[:].bitcast(i32), 1, None,
        op0=OP.logical_shift_right,
    )
    nc.vector.tensor_scalar(
        yy[:].bitcast(i32), yy[:].bitcast(i32), -1, RSQRT_MAGIC,
        op0=OP.mult, op1=OP.add,
    )
    for _ in range(3):
        y2 = small.tile([BC, O], f32, tag="y2")
        nc.vector.tensor_mul(y2[:], yy[:], yy[:])
        t2 = small.tile([BC, O], f32, tag="t2")
        nc.vector.tensor_mul(t2[:], y2[:], dd[:])
        u2 = small.tile([BC, O], f32, tag="u2")
        nc.vector.tensor_scalar(
            u2[:], t2[:], -0.5, 1.5, op0=OP.mult, op1=OP.add
        )
        yn = small.tile([BC, O], f32, tag="yn")
        nc.vector.tensor_mul(yn[:], yy[:], u2[:])
        yy = yn
    sc = small.tile([BC, O], f32, tag="sc")
    nc.vector.tensor_mul(sc[:], nsq[:], yy[:])
    v3 = small.tile([32, ON], f32, tag="v3")
    nc.vector.memset(v3[:], 0.0)
    nc.vector.tensor_mul(
        v3[0:BC, :].rearrange("b (o n) -> b o n", n=N),
        s3[:].rearrange("b (o n) -> b o n", n=N),
        sc[:].unsqueeze(2).broadcast_to([BC, O, N]),
    )

    if it == n_routing - 1:
        nc.scalar.dma_start(
            out=out_d[c * BC:(c + 1) * BC, :, :],
            in_=v3[0:BC, :].rearrange("b (o n) -> b o n", n=N),
        )
        return

    # replicate v across il: vrep[(b,il), (o,n)] = v[b,o,n]
    v3b = small.tile([32, ON], bf16, tag="v3b")
    nc.vector.tensor_copy(v3b[:], v3[:])
    vrep = small.tile([128, ON], bf16, tag="vrep")
    for q in range(4):
        nc.vector.stream_shuffle(
            vrep[q * 32:(q + 1) * 32, :],
            v3b[:],
            [2 * q + (j // 16) for j in range(32)],
        )
    # agreement a[(b,il), ib, o] = sum_n U*vrep, 2 halves, n-reduce as
    # a bf16 add-tree (tensor_reduce runs 1x-only, the tree gets 2x)
    H = IB // 2
    a_t = small.tile([128, IB, O], f32, tag="a_t")
    for h in range(2):
        ph = tree.tile([128, H, O, N], bf16, tag="prodh")
        nc.vector.tensor_mul(
            ph[:],
            U[:, h * H:(h + 1) * H, :].rearrange("p i (o n) -> p i o n", n=N),
            vrep[:]
            .rearrange("p (o n) -> p o n", n=N)
            .unsqueeze(1)
            .broadcast_to([128, H, O, N]),
        )
        t8 = tree.tile([128, H, O, 8], bf16, tag="t8")
        nc.vector.tensor_add(t8[:], ph[:, :, :, 0:8], ph[:, :, :, 8:16])
        t4 = tree.tile([128, H, O, 4], bf16, tag="t4")
        nc.vector.tensor_add(t4[:], t8[:, :, :, 0:4], t8[:, :, :, 4:8])
        t2t = tree.tile([128, H, O, 2], bf16, tag="t2t")
        nc.vector.tensor_add(t2t[:], t4[:, :, :, 0:2], t4[:, :, :, 2:4])
        nc.vector.tensor_add(
            a_t[:, h * H:(h + 1) * H, :], t2t[:, :, :, 0], t2t[:, :, :, 1]
        )
    if it == 0:
        nc.vector.tensor_copy(bbst[:], a_t[:])
        bbcur = a_t
    else:
        bb2 = small.tile([128, IB, O], f32, tag="bb2")
        nc.vector.tensor_add(bb2[:], bbst[:], a_t[:])
        if it < n_routing - 2:
            nc.vector.tensor_copy(bbst[:], bb2[:])
        bbcur = bb2

    # softmax over o -> c2n [128, O, IB] bf16, scatter to CBD
    c2 = small.tile([128, O, IB], f32, tag="c2")
    nc.scalar.activation(c2[:].transpose([0, 2, 1]), bbcur[:], AF.Exp)
    ssum = small.tile([128, IB], f32, tag="ssum")
    nc.vector.tensor_reduce(
        ssum[:], c2[:].transpose([0, 2, 1]), axis=AX.X, op=OP.add
    )
    rs = small.tile([128, IB], f32, tag="rs")
    nc.vector.reciprocal(rs[:], ssum[:])
    c2n = small.tile([128, O, IB], bf16, tag="c2n")
    nc.vector.tensor_mul(
        c2n[:], c2[:], rs[:].unsqueeze(1).broadcast_to([128, O, IB])
    )
    for b in range(BC):
        nc.sync.dma_start(
            out=cbd[b * 16:(b + 1) * 16, b:80:8, :],
            in_=c2n[b * 16:(b + 1) * 16, :, :],
        )


_CACHE = {}


def _get(n_routing: int):
    if n_routing not in _CACHE:
        _CACHE[n_routing] = _build(n_routing)
    return _CACHE[n_routing]


def _bf16(a):
    import ml_dtypes

    return np.asarray(a, dtype=ml_dtypes.bfloat16)


def _prep_host(inputs: np.ndarray, W: np.ndarray):
    x = np.ascontiguousarray(np.asarray(inputs, dtype=np.float32))
    W = np.asarray(W, dtype=np.float32)
    # w2[(il,v), ib, (o,n)] = W[ib*16+il, o, v, n]
    w2 = np.ascontiguousarray(
        W.reshape(IB, 16, O, V, N).transpose(1, 3, 0, 2, 4).reshape(128, IB, ON)
    )
    # e2[(b,il), (o,b')] = 0.1 * (b == b')   (uniform softmax weights)
    e2 = np.zeros((128, 80), dtype=np.float32)
    for b in range(8):
        e2[b * 16:(b + 1) * 16, np.arange(O) * 8 + b] = 0.1
    return x, _bf16(w2), _bf16(e2)


def _make_in_maps(inputs, W):
    x, w2, e2 = _prep_host(inputs, W)
    in_maps = []
    for core in range(NCORES):
        xc = x[core * BLOC:(core + 1) * BLOC]              # [32, 1152, 8]
        # xd[il, v, b, ib] = xc[b, ib*16+il, v]
        xdc = np.ascontiguousarray(
            _bf16(xc.reshape(BLOC, IB, 16, V).transpose(2, 3, 0, 1))
        )
        in_maps.append({"xd": xdc, "w2": w2, "e2": e2})
    return in_maps


def kernel(inputs, W, n_routing):
    from concourse.bass_utils import run_bass_kernel_spmd

    n_routing = int(n_routing)
    nc = _get(n_routing)
    in_maps = _make_in_maps(inputs, W)
    res = run_bass_kernel_spmd(nc, in_maps, core_ids=list(range(NCORES)))
    outs = [res.results[i]["out"] for i in range(NCORES)]
    return np.concatenate(outs, axis=0).astype(np.float32)
